# revision 39
# baseline (speedup 1.0000x reference)
"""Trainium2 Bass kernel for nn_NeuralFieldDiffusion (AdaLN DiT block).

Sharding (8 cores, fully SPMD-uniform program, per-core data differs):
  - Attention: head-parallel. Core i owns heads {2i, 2i+1} for BOTH batches.
  - proj / FFN / residuals: token-parallel. Core i owns flat tokens
    [512*i, 512*(i+1)) of the (B*N = 4096)-token stream.
  - One 8-way AllToAll reshards attention output (head-major -> token-major).
  - One 8-way AllGather distributes the (row-sharded) AdaLN modulation.

Everything on device is feature-major ([features on partitions, tokens on
free dim]) so every matmul contracts along the partition dim naturally.
Matmul inputs are bf16 (fp32 PSUM accumulation); the residual stream is fp32.
"""

import numpy as np
import ml_dtypes

import concourse.bass as bass
import concourse.mybir as mybir
import concourse.tile as tile
from concourse import bacc
from concourse.bass_utils import run_bass_kernel_spmd

F32 = mybir.dt.float32
BF16 = mybir.dt.bfloat16
AF = mybir.ActivationFunctionType
OP = mybir.AluOpType

HID = 1024
HEADS = 16
HD = 64
ROPE_DIM = 60
HALF_F = 10  # freqs per axis
SWIGLU = 2730
SWIGLU_P = 2816  # padded to 22*128
B = 2
N = 2048
TOK = B * N          # 4096 flat tokens
OWN = 512            # tokens owned per core
N_CORES = 8
EPS = 1e-6
THETA = 10000.0
GROUPS = [list(range(N_CORES))]
KC = HID // 128      # 8 k chunks
MI = SWIGLU_P // 128  # 22 ffn chunks

_cache = {}


def _patch_act_tables():
    """First-fit act-table assignment maps Ln->natural_log and
    Exp->exp_and_others, forcing a 1.28us table reload at every Ln/Exp
    transition (42 reloads in this kernel).  Strip those functions from
    every set other than the two we want resident so first-fit lands on
    natural_log_exp_and_others (ln+exp+copy+identity+square) for the whole
    middle of the kernel and silu_and_others (silu+sin+copy+identity) for
    the ends.  Set ids stay untouched so walrus still agrees with
    act_info.json."""
    if _cache.get("act_patched"):
        return
    _cache["act_patched"] = True
    orig = bacc.get_activation_tables

    def patched(arch):
        t = orig(arch)
        keep = ("natural_log_exp_and_others", "silu_and_others")
        covered = set()
        for name in keep:
            covered |= t[name]
        for name, fns in t.items():
            if name not in keep:
                t[name] = fns - covered
        return t

    bacc.get_activation_tables = patched


def _freqs():
    return 1.0 / THETA ** (np.arange(HALF_F, dtype=np.float64) / HALF_F)


def _perm():
    # head-dim permutation: rope-evens, passthrough dims, rope-odds
    return list(range(0, ROPE_DIM, 2)) + list(range(ROPE_DIM, HD)) + \
        list(range(1, ROPE_DIM, 2))


def build_program():
    _patch_act_tables()
    nc = bacc.Bacc("TRN2", target_bir_lowering=False, debug=False,
                   num_devices=N_CORES)

    # ---------------- dram I/O ----------------
    d_xt = nc.dram_tensor("xt", [HID, TOK], BF16, kind="ExternalInput")
    d_xt_own = nc.dram_tensor("xt_own", [HID, OWN], F32, kind="ExternalInput")
    d_qkw = nc.dram_tensor("qkw", [HID, 256], BF16, kind="ExternalInput")
    d_vw = nc.dram_tensor("vw", [HID, 130], BF16, kind="ExternalInput")
    d_projw = nc.dram_tensor("projw", [HID, HID], BF16, kind="ExternalInput")
    d_projb = nc.dram_tensor("projb", [128, 8], F32, kind="ExternalInput")
    d_w1 = nc.dram_tensor("w1w", [SWIGLU_P, HID], BF16, kind="ExternalInput")
    d_w3 = nc.dram_tensor("w3w", [SWIGLU_P, HID], BF16, kind="ExternalInput")
    d_w2 = nc.dram_tensor("w2w", [SWIGLU_P, HID], BF16, kind="ExternalInput")
    d_adaw = nc.dram_tensor("adaw", [HID, 768], BF16, kind="ExternalInput")
    d_adab = nc.dram_tensor("adab", [1, 768], F32, kind="ExternalInput")
    d_cvec = nc.dram_tensor("cvec", [128, 16], BF16, kind="ExternalInput")
    d_post = nc.dram_tensor("posT", [3, TOK], F32, kind="ExternalInput")
    d_fmat = nc.dram_tensor("fmat", [128, 128], F32, kind="ExternalInput")
    d_onespad = nc.dram_tensor("onespad", [128, 128], F32,
                               kind="ExternalInput")
    d_ident = nc.dram_tensor("ident", [128, 128], F32, kind="ExternalInput")
    d_pmat = nc.dram_tensor("pmat", [128, 128], BF16, kind="ExternalInput")
    d_ones128 = nc.dram_tensor("ones128", [1, 128], F32, kind="ExternalInput")
    d_ones128b = nc.dram_tensor("ones128b", [1, 128], BF16, kind="ExternalInput")
    d_onescol = nc.dram_tensor("onescol", [128, 1], F32, kind="ExternalInput")
    d_onescolb = nc.dram_tensor("onescolb", [128, 1], BF16,
                                kind="ExternalInput")
    d_qkmask = nc.dram_tensor("qkmask", [128, 8], BF16, kind="ExternalInput")
    d_selmask = nc.dram_tensor("selmask", [128, 256], BF16,
                               kind="ExternalInput")
    d_qsbias = nc.dram_tensor("qsbias", [4, 1], F32, kind="ExternalInput")
    d_msel = nc.dram_tensor("msel", [128, 2], F32, kind="ExternalInput")
    d_n2w = nc.dram_tensor("n2w", [128, 8], F32, kind="ExternalInput")
    d_out = nc.dram_tensor("out", [HID, OWN], F32, kind="ExternalOutput")

    with tile.TileContext(nc) as tc:
        _emit(nc, tc, locals())
    nc.compile()
    return nc


def _emit(nc, tc, d):
    d_xt = d["d_xt"]; d_xt_own = d["d_xt_own"]; d_qkw = d["d_qkw"]
    d_vw = d["d_vw"]; d_projw = d["d_projw"]; d_projb = d["d_projb"]
    d_w1 = d["d_w1"]; d_w3 = d["d_w3"]; d_w2 = d["d_w2"]
    d_adaw = d["d_adaw"]; d_adab = d["d_adab"]; d_cvec = d["d_cvec"]
    d_post = d["d_post"]; d_fmat = d["d_fmat"]; d_ones128 = d["d_ones128"]
    d_ones128b = d["d_ones128b"]; d_onescol = d["d_onescol"]
    d_onescolb = d["d_onescolb"]
    d_pmat = d["d_pmat"]; d_onespad = d["d_onespad"]
    d_ident = d["d_ident"]
    d_qkmask = d["d_qkmask"]; d_selmask = d["d_selmask"]
    d_qsbias = d["d_qsbias"]; d_msel = d["d_msel"]; d_n2w = d["d_n2w"]
    d_out = d["d_out"]

    sync = nc.sync
    act = nc.scalar
    dve = nc.vector
    pe = nc.tensor
    gps = nc.gpsimd

    with tc.tile_pool(name="dram", bufs=1, space="DRAM") as dram, \
         tc.tile_pool(name="p0", bufs=1) as p0, \
         tc.tile_pool(name="plate", bufs=1) as plate:

        # ---- dram bounce buffers for collectives ----
        ag_in = dram.tile([1536], F32)
        ag_out = dram.tile([1536 * N_CORES], F32, addr_space="Shared")
        a2_in = dram.tile([N_CORES, 128, OWN], BF16)
        a2_out = dram.tile([N_CORES, 128, OWN], BF16)

        # ---- small persistent constants ----
        cvec = p0.tile([128, 16], BF16)
        ones128 = p0.tile([1, 128], F32)
        ones128b = p0.tile([1, 128], BF16)
        onescol = p0.tile([128, 1], F32)
        onescolb = p0.tile([128, 1], BF16)
        sync.dma_start(out=onescolb[:], in_=d_onescolb[:])
        qkmask = p0.tile([128, 8], BF16)
        selmask = p0.tile([128, 256], BF16)
        onespad = p0.tile([128, 128], F32)
        sync.dma_start(out=onespad[:], in_=d_onespad[:])
        ident = p0.tile([128, 128], F32)
        sync.dma_start(out=ident[:], in_=d_ident[:])
        qsbias = p0.tile([4, 1], F32)
        msel = p0.tile([128, 2], F32)
        adab = p0.tile([1, 768], F32)
        projb = p0.tile([128, 8], F32)
        n2w = p0.tile([128, 8], F32)
        fmat = p0.tile([128, 128], F32)
        pmat = p0.tile([128, 128], BF16)
        sync.dma_start(out=pmat[:], in_=d_pmat[:])
        sync.dma_start(out=cvec[:], in_=d_cvec[:])
        sync.dma_start(out=ones128[:], in_=d_ones128[:])
        sync.dma_start(out=ones128b[:], in_=d_ones128b[:])
        sync.dma_start(out=onescol[:], in_=d_onescol[:])
        sync.dma_start(out=qkmask[:], in_=d_qkmask[:])
        sync.dma_start(out=selmask[:], in_=d_selmask[:])
        sync.dma_start(out=qsbias[:], in_=d_qsbias[:])
        sync.dma_start(out=msel[:], in_=d_msel[:])
        sync.dma_start(out=adab[:], in_=d_adab[:])
        sync.dma_start(out=projb[:], in_=d_projb[:])
        sync.dma_start(out=n2w[:], in_=d_n2w[:])
        sync.dma_start(out=fmat[:], in_=d_fmat[:])

        # const tiles for activation biases (must be APs for non-Copy funcs)
        cpi2 = p0.tile([128, 1], F32)
        dve.memset(cpi2[:], float(np.pi / 2))
        ceps = p0.tile([128, 1], F32)
        dve.memset(ceps[:], EPS)

        # persistent mid-size state
        modall = p0.tile([128, 96], F32)
        mod_own = p0.tile([128, 48], F32)
        sc1n_mlp = p0.tile([128, 8], F32)
        gbp = p0.tile([128, 8], F32)

        # ---- late-phase persistent tiles (proj/ffn) ----
        x2 = [plate.tile([128, OWN], F32, name=f"x2_{m}", tag=f"x2_{m}")
              for m in range(KC)]
        xh2 = [plate.tile([128, OWN], BF16, name=f"xh2_{m}", tag=f"xh2_{m}")
               for m in range(KC)]
        ofull = [plate.tile([128, OWN], BF16, name=f"of_{m}", tag=f"of_{m}")
                 for m in range(KC)]
        gated = [plate.tile([128, OWN], BF16, name=f"g_{mi}", tag=f"g_{mi}")
                 for mi in range(MI)]

        def modcol(b, comp, cf):
            cc = 8 * comp + cf
            return 12 * (cc // 6) + 6 * b + (cc % 6)

        # =======================================================
        # Phase A: sin/cos, AdaLN matmul + AllGather
        # =======================================================
        with tc.tile_pool(name="p_att", bufs=1) as p_att:
            # q/k tiles (post-qkv, per batch) and v_tok tiles
            qt = [p_att.tile([128, N], BF16, name=f"qt{b}", tag=f"qt{b}")
                  for b in range(B)]
            kt = [p_att.tile([128, N], BF16, name=f"kt{b}", tag=f"kt{b}")
                  for b in range(B)]
            ktp1 = [p_att.tile([128, N], BF16, name=f"ktp1{b}",
                               tag=f"ktp1{b}") for b in range(B)]
            vtok = [[p_att.tile([128, 130], BF16, name=f"vt{b}_{t}",
                                tag=f"vt{b}_{t}") for t in range(16)]
                    for b in range(B)]

            with tc.tile_pool(name="p_ph1", bufs=1) as p_ph1, \
                 tc.psum_pool(name="ps1", bufs=1) as ps1:

                # sincos tiles (consumed by rope, freed before attention)
                sinf = [p_ph1.tile([128, N], BF16, name=f"sinf{b}",
                                   tag=f"sinf{b}") for b in range(B)]
                cosf = [p_ph1.tile([128, N], BF16, name=f"cosf{b}",
                                   tag=f"cosf{b}") for b in range(B)]

                # --- sin/cos (Sin table set loads first) ---
                for b in range(B):
                    for t in range(4):
                        pts = p_ph1.tile([128, 512], F32, tag="pts", bufs=2)
                        sync.dma_start(out=pts[0:3, :], in_=d_post[
                            :, b * N + 512 * t: b * N + 512 * (t + 1)])
                        ang = ps1.tile([128, 512], F32, tag="mm", bufs=4)
                        pe.matmul(ang[:], fmat[:], pts[:],
                                  start=True, stop=True)
                        sl = slice(512 * t, 512 * (t + 1))
                        act.activation(sinf[b][:, sl], ang[:], AF.Sin)
                        act.activation(cosf[b][:, sl], ang[:], AF.Sin,
                                       bias=cpi2[:])

                # --- AdaLN: mod slice = adaw.T @ c (per batch) + bias ---
                # AdaLN computed transposed: out[p, 6b+c] so the
                # AllGather payload is partition-major (no transpose DMAs).
                msl = p_ph1.tile([1, 1536], F32, tag="msl")
                ada_ps = [[ps1.tile([1, 512], F32, name=f"adaps{b}_{n}",
                                    tag="row", bufs=4)
                           for n in range(2)] for b in range(B)]
                with tc.tile_pool(name="p_ada", bufs=1) as p_ada:
                    adaw_ring = [p_ada.tile([128, 768], BF16, name=f"adaw{k}",
                                            tag="adaw", bufs=3)
                                 for k in range(KC)]
                    for k in range(KC):
                        sync.dma_start(out=adaw_ring[k][:],
                                       in_=d_adaw[128 * k:128 * (k + 1), :])
                    for b in range(B):
                        for n in range(2):
                            lo, sz = (0, 512) if n == 0 else (512, 256)
                            for k in range(KC):
                                pe.matmul(ada_ps[b][n][:, 0:sz],
                                          cvec[:, 2 * k + b: 2 * k + b + 1],
                                          adaw_ring[k][:, lo:lo + sz],
                                          start=(k == 0), stop=(k == KC - 1))
                            dve.tensor_tensor(
                                out=msl[:, 768 * b + lo: 768 * b + lo + sz],
                                in0=ada_ps[b][n][:, 0:sz],
                                in1=adab[:, lo:lo + sz], op=OP.add)
                sync.dma_start(out=ag_in[:], in_=msl[:])
                gps.collective_compute(
                    "AllGather", OP.bypass, replica_groups=GROUPS,
                    ins=[ag_in.opt()], outs=[ag_out.opt()])
                sc1_b = []
                shb_b = []

                # =======================================================
                # Phase B (per batch): norm1 -> x_hat -> qkv -> rope
                # =======================================================
                qkw_sb = [p_ph1.tile([128, 256], BF16, name=f"qkw{k}",
                                     tag=f"qkw{k}") for k in range(KC)]
                vw_sb = [p_ph1.tile([128, 130], BF16, name=f"vw{k}",
                                    tag=f"vw{k}") for k in range(KC)]
                for k in range(KC):
                    sync.dma_start(out=qkw_sb[k][:],
                                   in_=d_qkw[128 * k:128 * (k + 1), :])
                    sync.dma_start(out=vw_sb[k][:],
                                   in_=d_vw[128 * k:128 * (k + 1), :])

                for b in range(B):
                    with tc.tile_pool(name=f"p_x{b}", bufs=1) as p_x:
                        xhat = [p_x.tile([128, N], BF16, name=f"xh{b}_{f}",
                                         tag=f"xh_{f}") for f in range(KC)]
                        # norm1 over this batch's 2048 tokens, 512 at a time
                        for t in range(4):
                            gsl = slice(b * N + 512 * t, b * N + 512 * (t + 1))
                            lsl = slice(512 * t, 512 * (t + 1))
                            xts = [p_x.tile([128, 512], BF16, name=f"xt_{f}",
                                            tag="xtr", bufs=9)
                                   for f in range(KC)]
                            ss = ps1.tile([1, 512], F32, tag="row", bufs=4)
                            for f in range(KC):
                                eng = sync if f % 2 == 0 else act
                                eng.dma_start(out=xts[f][:], in_=d_xt[
                                    128 * f:128 * (f + 1), gsl])
                                sq = p_x.tile([128, 512], BF16, tag="sq",
                                              bufs=2)
                                dve.tensor_tensor(out=sq[:], in0=xts[f][:],
                                                  in1=xts[f][:], op=OP.mult)
                                pe.matmul(ss[:], onescolb[:], sq[:],
                                          start=(f == 0), stop=(f == KC - 1))
                            lnv = p_x.tile([1, 512], F32, tag="lnv", bufs=1)
                            act.activation(lnv[:], ss[:], AF.Ln,
                                           bias=ceps[0:1, :],
                                           scale=float(1.0 / HID))
                            rstd = p_x.tile([128, 512], F32, tag="rstd",
                                            bufs=1)
                            act.activation(rstd[0:1, :], lnv[:], AF.Exp,
                                           scale=-0.5)
                            bc_ps = ps1.tile([128, 512], F32, tag="mm", bufs=4)
                            pe.matmul(bc_ps[:], onespad[:], rstd[:],
                                      start=True, stop=True)
                            bc = p_x.tile([128, 512], BF16, tag="bc", bufs=2)
                            act.activation(bc[:], bc_ps[:], AF.Copy)
                            for f in range(KC):
                                dve.tensor_tensor(out=xhat[f][:, lsl],
                                                  in0=xts[f][:], in1=bc[:],
                                                  op=OP.mult)

                        if b == 0:
                            # mod swizzle deferred to here: batch-0 norm1 PE
                            # work above covers the AllGather latency.
                            ag4 = ag_out.rearrange("(q x p) -> q x p",
                                                   q=N_CORES, x=12)
                            for q in range(N_CORES):
                                t12 = p_ph1.tile([12, 128], F32, tag="t12",
                                                 bufs=2)
                                sync.dma_start(out=t12[:], in_=ag4[q])
                                tps = ps1.tile([128, 12], F32, tag="row",
                                               bufs=4)
                                pe.transpose(tps[:], t12[:], ident[0:12, 0:12])
                                dve.tensor_copy(
                                    out=modall[:, 12 * q:12 * (q + 1)],
                                    in_=tps[:])

                            # mod_own = blend of batch columns by msel
                            t_b0 = p_ph1.tile([128, 48], F32, tag="t_b0")
                            mv = modall.rearrange("p (q x) -> p q x", q=N_CORES)
                            for bb in range(B):
                                srcb = mv[:, :, 6 * bb:6 * bb + 6]
                                if bb == 0:
                                    dve.tensor_scalar(
                                        t_b0[:].rearrange("p (q x) -> p q x",
                                                          q=8),
                                        srcb, msel[:, 0:1], None, OP.mult)
                                else:
                                    dve.tensor_scalar(
                                        mod_own[:].rearrange("p (q x) -> p q x",
                                                             q=8),
                                        srcb, msel[:, 1:2], None, OP.mult)
                            dve.tensor_tensor(out=mod_own[:], in0=mod_own[:],
                                              in1=t_b0[:], op=OP.add)
                            dve.tensor_scalar(sc1n_mlp[:], mod_own[:, 32:40],
                                              1.0, None, OP.add)
                            dve.tensor_tensor(out=sc1n_mlp[:], in0=sc1n_mlp[:],
                                              in1=n2w[:], op=OP.mult)
                            dve.tensor_tensor(out=gbp[:], in0=mod_own[:, 16:24],
                                              in1=projb[:], op=OP.mult)

                            for bb in range(B):
                                sv = p_ph1.tile([128, 8], F32,
                                                name=f"sc1_{bb}",
                                                tag=f"sc1_{bb}")
                                hv = p_ph1.tile([128, 8], BF16,
                                                name=f"shb_{bb}",
                                                tag=f"shb_{bb}")
                                for cf in range(KC):
                                    c_sc = modcol(bb, 1, cf)
                                    dve.tensor_scalar(sv[:, cf:cf + 1],
                                                      modall[:, c_sc:c_sc + 1],
                                                      1.0, None, OP.add)
                                    c_sh = modcol(bb, 0, cf)
                                    dve.tensor_copy(
                                        out=hv[:, cf:cf + 1],
                                        in_=modall[:, c_sh:c_sh + 1])
                                sc1_b.append(sv)
                                shb_b.append(hv)

                        # ---- fold (1+sc)*norm1_w into weights (per batch) ----
                        qkw_b = [p_x.tile([128, 256], BF16, name=f"qkwb{k}",
                                          tag=f"qkwb{k}") for k in range(KC)]
                        vw_b = [p_x.tile([128, 130], BF16, name=f"vwb{k}",
                                         tag=f"vwb{k}") for k in range(KC)]
                        for k in range(KC):
                            dve.tensor_scalar(qkw_b[k][:], qkw_sb[k][:],
                                              sc1_b[b][:, k:k + 1], None, OP.mult)
                            dve.tensor_scalar(vw_b[k][:], vw_sb[k][:],
                                              sc1_b[b][:, k:k + 1], None, OP.mult)

                        # ---- biases (use unscaled weights x sh) ----
                        qkb = p_x.tile([128, 2], F32, tag="qkb")
                        for m in range(2):
                            bps = ps1.tile([128, 1], F32, tag="mm", bufs=4)
                            for k in range(KC):
                                pe.matmul(bps[:], qkw_sb[k][:, 128 * m:128 * (m + 1)],
                                          shb_b[b][:, k:k + 1],
                                          start=(k == 0), stop=(k == KC - 1))
                            act.activation(qkb[:, m:m + 1], bps[:], AF.Copy)
                        vbp = ps1.tile([1, 130], F32, tag="row", bufs=4)
                        for k in range(KC):
                            pe.matmul(vbp[:], shb_b[b][:, k:k + 1], vw_sb[k][:],
                                      start=(k == 0), stop=(k == KC - 1))
                        vb_sb = p_x.tile([1, 130], BF16, tag="vb_sb")
                        act.activation(vb_sb[:], vbp[:], AF.Copy)

                        # ---- q/k matmuls ----
                        for m in range(2):
                            dst = qt[b] if m == 0 else kt[b]
                            for t in range(4):
                                ps = ps1.tile([128, 512], F32, tag="mm", bufs=4)
                                for k in range(KC):
                                    pe.matmul(
                                        ps[:],
                                        qkw_b[k][:, 128 * m:128 * (m + 1)],
                                        xhat[k][:, 512 * t:512 * (t + 1)],
                                        start=(k == 0), stop=(k == KC - 1))
                                act.activation(dst[:, 512 * t:512 * (t + 1)],
                                               ps[:], AF.Identity,
                                               bias=qkb[:, m:m + 1])

                        # ---- v_tok (token-major v with ones column) ----
                        for tb in range(16):
                            ps = ps1.tile([128, 130], F32, tag="mm", bufs=4)
                            for k in range(KC):
                                pe.matmul(ps[:], xhat[k][:, 128 * tb:128 * (tb + 1)],
                                          vw_b[k][:], start=(k == 0), stop=False)
                            pe.matmul(ps[:], ones128b[:], vb_sb[:],
                                      start=False, stop=True)
                            dve.tensor_copy(out=vtok[b][tb][:], in_=ps[:])
                            vv = vtok[b][tb].rearrange("p (h x) -> p h x", h=2)
                            dve.memset(vv[:, :, 64:65], 1.0)

                        # ---- q/k rmsnorm (joint, partition-reduced) ----
                        rstd8 = p_x.tile([128, N], BF16, tag="rstd8")
                        dve.memset(rstd8[:], 0.0)
                        ss8 = [ps1.tile([4, 512], F32, name=f"ss8_{t}",
                                        tag="row", bufs=4) for t in range(4)]
                        for m in range(2):
                            src = qt[b] if m == 0 else kt[b]
                            for t in range(4):
                                sqm = p_x.tile([128, 512], BF16, tag="sqm",
                                               bufs=2)
                                sl = slice(512 * t, 512 * (t + 1))
                                dve.tensor_tensor(out=sqm[:], in0=src[:, sl],
                                                  in1=src[:, sl], op=OP.mult)
                                pe.matmul(ss8[t][:],
                                          qkmask[:, 4 * m:4 * (m + 1)],
                                          sqm[:],
                                          start=(m == 0), stop=(m == 1))
                        for t in range(4):
                            l8 = p_x.tile([4, 512], F32, tag="l8", bufs=1)
                            act.activation(l8[:], ss8[t][:], AF.Ln,
                                           bias=ceps[0:4, :],
                                           scale=float(1.0 / HD))
                            act.activation(rstd8[0:4, 512 * t:512 * (t + 1)],
                                           l8[:], AF.Exp, scale=-0.5,
                                           bias=qsbias[:])
                        # apply rstd (* q_norm_w/k_norm_w via selmask)
                        for m in range(2):
                            dst = qt[b] if m == 0 else kt[b]
                            for t in range(4):
                                bcp = ps1.tile([128, 512], F32, tag="mm", bufs=4)
                                pe.matmul(bcp[:],
                                          selmask[:, 128 * m:128 * (m + 1)],
                                          rstd8[:, 512 * t:512 * (t + 1)],
                                          start=True, stop=True)
                                sl = slice(512 * t, 512 * (t + 1))
                                dve.tensor_tensor(out=dst[:, sl], in0=dst[:, sl],
                                                  in1=bcp[:], op=OP.mult)

                        # ---- rope (partner shuffle via PE permutation) ----
                        for m in range(2):
                            src = qt[b] if m == 0 else kt[b]
                            for t in range(4):
                                sl = slice(512 * t, 512 * (t + 1))
                                pps = ps1.tile([128, 512], F32, tag="mm",
                                               bufs=4)
                                pe.matmul(pps[:], pmat[:], src[:, sl],
                                          start=True, stop=True)
                                part = p_x.tile([128, 512], BF16, tag="part",
                                                bufs=2)
                                dve.tensor_tensor(out=part[:], in0=pps[:],
                                                  in1=sinf[b][:, sl],
                                                  op=OP.mult)
                                dve.tensor_tensor(out=src[:, sl],
                                                  in0=src[:, sl],
                                                  in1=cosf[b][:, sl],
                                                  op=OP.mult)
                                dve.tensor_tensor(out=src[:, sl],
                                                  in0=src[:, sl],
                                                  in1=part[:], op=OP.add)

                        # zero-padded key tiles: scores contract K=128 with
                        # the unused half of the array masked by zero rows
                        # (K<=64 matmuls run at half rate and never ramp).
                        dve.tensor_copy(out=ktp1[b][64:128, :],
                                        in_=kt[b][64:128, :])
                        dve.memset(ktp1[b][0:64, :], 0.0)
                        dve.memset(kt[b][64:128, :], 0.0)

            # =======================================================
            # Phase C: attention (per batch, per local head, per half)
            # =======================================================
            with tc.tile_pool(name="p_att2", bufs=1) as p_at2:
                psA = tc.alloc_tile_pool(name="ps_att", bufs=1, space="PSUM")

                # prefetch FFN w1/w3 as mi-major rings; transfers hide
                # under attention, ring refills during the gated loop
                w1t = [p_at2.tile([128, HID], BF16, name=f"w1_{mi}",
                                  tag="w1r", bufs=8) for mi in range(MI)]
                w3t = [p_at2.tile([128, HID], BF16, name=f"w3_{mi}",
                                  tag="w3r", bufs=8) for mi in range(MI)]
                for mi in range(8):
                    sync.dma_start(out=w1t[mi][:],
                                   in_=d_w1[128 * mi:128 * (mi + 1), :])
                    sync.dma_start(out=w3t[mi][:],
                                   in_=d_w3[128 * mi:128 * (mi + 1), :])

                def normalize(oT, b, hl, half):
                    # softmax normalize: divide rows 0-63 by the ones-row
                    # (row 64).  DVE reciprocal + PE broadcast; no ACT work
                    # so the Exp stream of the next block is undisturbed.
                    hsl = slice(64 * hl, 64 * (hl + 1))
                    rinv = p_at2.tile([128, 1024], F32, tag="rinv", bufs=2)
                    dve.reciprocal(rinv[0:1, :], oT[64:65, :])
                    bcp = psA.tile([64, 1024], F32, tag="sc", bufs=2)
                    for q2 in range(2):
                        pe.matmul(bcp[:, 512 * q2:512 * (q2 + 1)],
                                  onespad[:, 0:64],
                                  rinv[:, 512 * q2:512 * (q2 + 1)],
                                  start=True, stop=True)
                    bc_sb = p_at2.tile([64, 1024], F32, tag="bcs", bufs=2)
                    dve.tensor_copy(out=bc_sb[:], in_=bcp[:])
                    osb = p_at2.tile([64, 1024], BF16, tag="osb", bufs=2)
                    dve.tensor_tensor(out=osb[:], in0=oT[0:64, :],
                                      in1=bc_sb[:], op=OP.mult)
                    for s2 in range(2):
                        sync.dma_start(
                            out=a2_in[4 * b + 2 * half + s2, hsl, :],
                            in_=osb[:, 512 * s2:512 * (s2 + 1)])

                pending = None
                for b in range(B):
                    for hl in range(2):
                        hsl = slice(64 * hl, 64 * (hl + 1))
                        for half in range(2):
                            oT = psA.tile([65, 1024], F32, tag="oT", bufs=2)
                            for nk in range(16):
                                if nk == 6 and pending is not None:
                                    normalize(*pending)
                                    pending = None
                                sc = psA.tile([128, 1024], F32, tag="sc", bufs=2)
                                ksrc = kt[b] if hl == 0 else ktp1[b]
                                for q2 in range(2):
                                    qsl = slice(1024 * half + 512 * q2,
                                                1024 * half + 512 * (q2 + 1))
                                    pe.matmul(sc[:, 512 * q2:512 * (q2 + 1)],
                                              ksrc[:, 128 * nk:128 * (nk + 1)],
                                              qt[b][:, qsl],
                                              start=True, stop=True)
                                at = p_at2.tile([128, 1024], BF16, tag="at",
                                                bufs=3)
                                act.activation(at[:], sc[:], AF.Exp)
                                for q2 in range(2):
                                    pe.matmul(
                                        oT[:, 512 * q2:512 * (q2 + 1)],
                                        vtok[b][nk][:, 65 * hl:65 * (hl + 1)],
                                        at[:, 512 * q2:512 * (q2 + 1)],
                                        start=(nk == 0), stop=(nk == 15))
                            pending = (oT, b, hl, half)
                normalize(*pending)

                # ---- AllToAll: head-major -> token-major ----
                gps.collective_compute(
                    "AllToAll", OP.bypass, replica_groups=GROUPS,
                    ins=[a2_in.opt()], outs=[a2_out.opt()])
                for k in range(KC):
                    sync.dma_start(out=ofull[k][:], in_=a2_out[k])

                # ---- proj + gated residual -> x2 (fp32) ----
                projw_sb = [p_at2.tile([128, HID], BF16, name=f"pw_{k}",
                                       tag=f"pw_{k}") for k in range(KC)]
                for m in range(KC):
                    sync.dma_start(out=x2[m][:],
                                   in_=d_xt_own[128 * m:128 * (m + 1), :])
                for k in range(KC):
                    sync.dma_start(out=projw_sb[k][:],
                                   in_=d_projw[128 * k:128 * (k + 1), :])
                psA.release()
                with tc.psum_pool(name="ps_pr", bufs=1) as psP:
                    for m in range(KC):
                        ps = psP.tile([128, 512], F32, tag="mm", bufs=4)
                        for k in range(KC):
                            pe.matmul(ps[:], projw_sb[k][:, 128 * m:128 * (m + 1)],
                                      ofull[k][:], start=(k == 0),
                                      stop=(k == KC - 1))
                        tg = p_at2.tile([128, OWN], F32, tag="tg", bufs=2)
                        act.activation(tg[:], ps[:], AF.Identity,
                                       bias=gbp[:, m:m + 1],
                                       scale=mod_own[:, 16 + m:17 + m])
                        dve.tensor_tensor(out=x2[m][:], in0=x2[m][:],
                                          in1=tg[:], op=OP.add)

                    # ---- norm2 + modulate -> xh2 (bf16) ----
                    ss2 = psP.tile([1, 512], F32, tag="row", bufs=2)
                    for m in range(KC):
                        sq = p_at2.tile([128, OWN], F32, tag="sq2", bufs=2)
                        dve.tensor_tensor(out=sq[:], in0=x2[m][:], in1=x2[m][:],
                                          op=OP.mult)
                        pe.matmul(ss2[:], onescol[:], sq[:],
                                  start=(m == 0), stop=(m == KC - 1))
                    ln2 = p_at2.tile([1, 512], F32, tag="ln2")
                    act.activation(ln2[:], ss2[:], AF.Ln, bias=ceps[0:1, :],
                                   scale=float(1.0 / HID))
                    rstd2 = p_at2.tile([128, 512], F32, tag="rstd2")
                    act.activation(rstd2[0:1, :], ln2[:], AF.Exp, scale=-0.5)
                    bc2p = psP.tile([128, 512], F32, tag="mm", bufs=4)
                    pe.matmul(bc2p[:], onespad[:], rstd2[:], start=True,
                              stop=True)
                    bc2 = p_at2.tile([128, 512], F32, tag="bc2")
                    act.activation(bc2[:], bc2p[:], AF.Copy)
                    for m in range(KC):
                        tmp = p_at2.tile([128, OWN], F32, tag="tmp2", bufs=2)
                        dve.tensor_tensor(out=tmp[:], in0=x2[m][:], in1=bc2[:],
                                          op=OP.mult)
                        dve.tensor_scalar(xh2[m][:], tmp[:],
                                          sc1n_mlp[:, m:m + 1],
                                          mod_own[:, 24 + m:25 + m],
                                          OP.mult, OP.add)

                    # ---- FFN w1/w3 + swiglu gate (weights prefetched) ----
                    for mi in range(MI):
                        if mi + 8 < MI:
                            sync.dma_start(
                                out=w1t[mi + 8][:],
                                in_=d_w1[128 * (mi + 8):128 * (mi + 9), :])
                            sync.dma_start(
                                out=w3t[mi + 8][:],
                                in_=d_w3[128 * (mi + 8):128 * (mi + 9), :])
                        p1 = psP.tile([128, 512], F32, tag="mm", bufs=4)
                        p3 = psP.tile([128, 512], F32, tag="mm", bufs=4)
                        for k in range(KC):
                            pe.matmul(p1[:],
                                      w1t[mi][:, 128 * k:128 * (k + 1)],
                                      xh2[k][:], start=(k == 0),
                                      stop=(k == KC - 1))
                        for k in range(KC):
                            pe.matmul(p3[:],
                                      w3t[mi][:, 128 * k:128 * (k + 1)],
                                      xh2[k][:], start=(k == 0),
                                      stop=(k == KC - 1))
                        s1 = p_at2.tile([128, OWN], BF16, tag="s1", bufs=3)
                        act.activation(s1[:], p1[:], AF.Silu)
                        dve.tensor_tensor(out=gated[mi][:], in0=s1[:],
                                          in1=p3[:], op=OP.mult)

        # =======================================================
        # Phase D: w2 + final residual
        # =======================================================
        with tc.tile_pool(name="p_ffn", bufs=1) as p_ffn:
            with tc.psum_pool(name="ps_f2", bufs=1) as psW:
                ffp = [psW.tile([128, 512], F32, name=f"ff_{m}", tag=f"ff_{m}")
                       for m in range(KC)]
                w2r = [p_ffn.tile([128, HID], BF16, name=f"w2_{mi}", tag="w2r",
                                  bufs=8) for mi in range(MI)]
                for mi in range(MI):
                    sync.dma_start(out=w2r[mi][:],
                                   in_=d_w2[128 * mi:128 * (mi + 1), :])
                    for m in range(KC):
                        pe.matmul(ffp[m][:], w2r[mi][:, 128 * m:128 * (m + 1)],
                                  gated[mi][:], start=(mi == 0),
                                  stop=(mi == MI - 1))
                for m in range(KC):
                    tg = p_ffn.tile([128, OWN], F32, tag="tgo", bufs=2)
                    act.activation(tg[:], ffp[m][:], AF.Identity, bias=0.0,
                                   scale=mod_own[:, 40 + m:41 + m])
                    outm = p_ffn.tile([128, OWN], F32, tag="outm", bufs=2)
                    dve.tensor_tensor(out=outm[:], in0=tg[:], in1=x2[m][:],
                                      op=OP.add)
                    sync.dma_start(out=d_out[128 * m:128 * (m + 1), :],
                                   in_=outm[:])


def _host_prep(x, c, positions, norm1_w, qkv_w, q_norm_w, k_norm_w, proj_w,
               proj_b, norm2_w, w1, w3, w2, ada_w, ada_b):
    xf = np.asarray(x, np.float32).reshape(TOK, HID)
    xt = np.ascontiguousarray(xf.T)                      # [1024, 4096]
    xtb = xt.astype(ml_dtypes.bfloat16)
    perm = np.array(_perm())
    freqs = _freqs()

    qkv_r = np.asarray(qkv_w, np.float32).reshape(3, HEADS, HD, HID)
    n1 = np.asarray(norm1_w, np.float32)
    # fmat: [128, 128] angle matrix (rows 0-2 live, rest zero so the
    # device-side rhs tiles only need their top rows filled)
    fmat = np.zeros((128, 128), np.float32)
    for r in range(128):
        rr = r % 64
        if rr < 30:
            j = rr
            fmat[j // 10, r] = -freqs[j % 10]
        elif rr >= 34:
            j = rr - 34
            fmat[j // 10, r] = freqs[j % 10]
    posT = np.ascontiguousarray(
        np.asarray(positions, np.float32).reshape(TOK, 3).T)

    # rope partner-shuffle as a PE permutation: out[d] = src[perm_src(d)]
    # lhsT layout: pmat[src_row, dst_row] = 1
    pmat = np.zeros((128, 128), np.float32)
    for dlo, slo, cnt in ((0, 34, 30), (34, 0, 30), (64, 98, 30),
                          (98, 64, 30), (30, 30, 4), (94, 94, 4)):
        for r in range(cnt):
            pmat[slo + r, dlo + r] = 1.0

    qkmask = np.zeros((128, 8), np.float32)
    qkmask[0:64, 0] = 1.0
    qkmask[64:128, 1] = 1.0
    qkmask[0:64, 6] = 1.0
    qkmask[64:128, 7] = 1.0
    # selmask carries q_norm_w / k_norm_w (permuted) in place of 1.0
    qnw = np.asarray(q_norm_w, np.float32)[perm]
    knw = np.asarray(k_norm_w, np.float32)[perm]
    selmask = np.zeros((128, 256), np.float32)
    for p in range(128):
        selmask[p // 64, p] = qnw[p % 64]
        selmask[2 + p // 64, 128 + p] = knw[p % 64]
    qsbias = np.array([[np.log(HD ** -0.5)]] * 2 + [[0.0]] * 2, np.float32)

    projw = np.ascontiguousarray(
        np.asarray(proj_w, np.float32).T * 1.0).astype(ml_dtypes.bfloat16)
    projb = np.asarray(proj_b, np.float32).reshape(8, 128).T.copy()
    n2w = (np.asarray(norm2_w, np.float32)).reshape(8, 128).T.copy()

    w1p = np.zeros((SWIGLU_P, HID), np.float32)
    w1p[:SWIGLU] = np.asarray(w1, np.float32)
    w3p = np.zeros((SWIGLU_P, HID), np.float32)
    w3p[:SWIGLU] = np.asarray(w3, np.float32)
    w2p = np.zeros((SWIGLU_P, HID), np.float32)
    w2p[:SWIGLU] = np.asarray(w2, np.float32).T
    def mi_major(w):  # [SWIGLU_P, HID] -> tile[mi][p, 128k+j] = w[128mi+j, 128k+p]
        w4 = w.reshape(MI, 128, KC, 128)          # [mi, j, k, p]
        w4 = np.ascontiguousarray(w4.transpose(0, 3, 2, 1))  # [mi, p, k, j]
        return w4.reshape(SWIGLU_P, HID)

    w1w = mi_major(w1p).astype(ml_dtypes.bfloat16)
    w3w = mi_major(w3p).astype(ml_dtypes.bfloat16)
    w2w = w2p.astype(ml_dtypes.bfloat16)

    adaw_t = np.ascontiguousarray(np.asarray(ada_w, np.float32).T)  # [1024, 6144]
    adab = np.asarray(ada_b, np.float32)
    cvec = np.zeros((128, 16), np.float32)
    cf = np.asarray(c, np.float32)
    for k in range(KC):
        for b in range(B):
            cvec[:, 2 * k + b] = cf[b, 128 * k:128 * (k + 1)]

    ones128 = np.ones((1, 128), np.float32)
    onescol = np.ones((128, 1), np.float32)
    onespad = np.zeros((128, 128), np.float32)
    onespad[0, :] = 1.0
    ident = np.eye(128, dtype=np.float32)

    common = dict(
        xt=xtb,
        projw=projw, projb=projb, n2w=n2w,
        w1w=w1w, w3w=w3w, w2w=w2w,
        cvec=cvec.astype(ml_dtypes.bfloat16),
        posT=posT, fmat=fmat, pmat=pmat.astype(ml_dtypes.bfloat16),
        ones128=ones128, ones128b=ones128.astype(ml_dtypes.bfloat16),
        onescol=onescol, onescolb=onescol.astype(ml_dtypes.bfloat16),
        onespad=onespad, ident=ident,
        qkmask=qkmask.astype(ml_dtypes.bfloat16),
        selmask=selmask.astype(ml_dtypes.bfloat16), qsbias=qsbias,
    )

    in_maps = []
    for i in range(N_CORES):
        m = dict(common)
        h0 = 2 * i
        q = qkv_r[0, h0:h0 + 2][:, perm, :]       # [2, 64, 1024]
        k = qkv_r[1, h0:h0 + 2][:, perm, :]
        v = qkv_r[2, h0:h0 + 2]                   # unpermuted
        qk = np.concatenate([q.reshape(128, HID), k.reshape(128, HID)], 0)
        qk = qk * n1[None, :]                     # fold norm1_w
        m["qkw"] = np.ascontiguousarray(qk.T).astype(ml_dtypes.bfloat16)
        vw = np.zeros((HID, 130), np.float32)
        for hh in range(2):
            vw[:, 65 * hh:65 * hh + 64] = (v[hh] * n1[None, :]).T
        m["vw"] = vw.astype(ml_dtypes.bfloat16)
        m["adaw"] = adaw_t[:, 768 * i:768 * (i + 1)].astype(ml_dtypes.bfloat16)
        m["adab"] = adab[768 * i:768 * (i + 1)].reshape(1, 768).copy()
        m["xt_own"] = np.ascontiguousarray(xt[:, OWN * i:OWN * (i + 1)])
        msel = np.zeros((128, 2), np.float32)
        own_batch = (OWN * i) // N
        msel[:, 0] = 1.0 - own_batch
        msel[:, 1] = float(own_batch)
        m["msel"] = msel
        in_maps.append(m)
    return in_maps


def kernel(**inputs) -> np.ndarray:
    if "nc" not in _cache:
        _cache["nc"] = build_program()
    nc = _cache["nc"]
    in_maps = _host_prep(**inputs)
    res = run_bass_kernel_spmd(nc, in_maps, core_ids=list(range(N_CORES)))
    out = np.empty((TOK, HID), np.float32)
    for i in range(N_CORES):
        out[OWN * i:OWN * (i + 1), :] = res.results[i]["out"].T
    return out.reshape(B, N, HID)



# revision 41
# speedup vs baseline: 1.0234x; 1.0234x over previous
"""Trainium2 Bass kernel for nn_NeuralFieldDiffusion (AdaLN DiT block).

Sharding (8 cores, fully SPMD-uniform program, per-core data differs):
  - Attention: head-parallel. Core i owns heads {2i, 2i+1} for BOTH batches.
  - proj / FFN / residuals: token-parallel. Core i owns flat tokens
    [512*i, 512*(i+1)) of the (B*N = 4096)-token stream.
  - One 8-way AllToAll reshards attention output (head-major -> token-major).
  - One 8-way AllGather distributes the (row-sharded) AdaLN modulation.

Everything on device is feature-major ([features on partitions, tokens on
free dim]) so every matmul contracts along the partition dim naturally.
Matmul inputs are bf16 (fp32 PSUM accumulation); the residual stream is fp32.
"""

import numpy as np
import ml_dtypes

import concourse.bass as bass
import concourse.mybir as mybir
import concourse.tile as tile
from concourse import bacc
from concourse.bass_utils import run_bass_kernel_spmd

F32 = mybir.dt.float32
BF16 = mybir.dt.bfloat16
AF = mybir.ActivationFunctionType
OP = mybir.AluOpType

HID = 1024
HEADS = 16
HD = 64
ROPE_DIM = 60
HALF_F = 10  # freqs per axis
SWIGLU = 2730
SWIGLU_P = 2816  # padded to 22*128
B = 2
N = 2048
TOK = B * N          # 4096 flat tokens
OWN = 512            # tokens owned per core
N_CORES = 8
EPS = 1e-6
THETA = 10000.0
GROUPS = [list(range(N_CORES))]
KC = HID // 128      # 8 k chunks
MI = SWIGLU_P // 128  # 22 ffn chunks

_cache = {}


def _patch_act_tables():
    """First-fit act-table assignment maps Ln->natural_log and
    Exp->exp_and_others, forcing a 1.28us table reload at every Ln/Exp
    transition (42 reloads in this kernel).  Strip those functions from
    every set other than the two we want resident so first-fit lands on
    natural_log_exp_and_others (ln+exp+copy+identity+square) for the whole
    middle of the kernel and silu_and_others (silu+sin+copy+identity) for
    the ends.  Set ids stay untouched so walrus still agrees with
    act_info.json."""
    if _cache.get("act_patched"):
        return
    _cache["act_patched"] = True
    orig = bacc.get_activation_tables

    def patched(arch):
        t = orig(arch)
        keep = ("natural_log_exp_and_others", "silu_and_others")
        covered = set()
        for name in keep:
            covered |= t[name]
        for name, fns in t.items():
            if name not in keep:
                t[name] = fns - covered
        return t

    bacc.get_activation_tables = patched


def _freqs():
    return 1.0 / THETA ** (np.arange(HALF_F, dtype=np.float64) / HALF_F)


def _perm():
    # head-dim permutation: rope-evens, passthrough dims, rope-odds
    return list(range(0, ROPE_DIM, 2)) + list(range(ROPE_DIM, HD)) + \
        list(range(1, ROPE_DIM, 2))


def build_program():
    _patch_act_tables()
    nc = bacc.Bacc("TRN2", target_bir_lowering=False, debug=False,
                   num_devices=N_CORES)

    # ---------------- dram I/O ----------------
    d_xt = nc.dram_tensor("xt", [HID, TOK], BF16, kind="ExternalInput")
    d_xt_own = nc.dram_tensor("xt_own", [HID, OWN], F32, kind="ExternalInput")
    d_qkw = nc.dram_tensor("qkw", [HID, 256], BF16, kind="ExternalInput")
    d_vw = nc.dram_tensor("vw", [HID, 130], BF16, kind="ExternalInput")
    d_projw = nc.dram_tensor("projw", [HID, HID], BF16, kind="ExternalInput")
    d_projb = nc.dram_tensor("projb", [128, 8], F32, kind="ExternalInput")
    d_w1 = nc.dram_tensor("w1w", [SWIGLU_P, HID], BF16, kind="ExternalInput")
    d_w3 = nc.dram_tensor("w3w", [SWIGLU_P, HID], BF16, kind="ExternalInput")
    d_w2 = nc.dram_tensor("w2w", [SWIGLU_P, HID], BF16, kind="ExternalInput")
    d_adaw = nc.dram_tensor("adaw", [HID, 768], BF16, kind="ExternalInput")
    d_adab = nc.dram_tensor("adab", [1, 768], F32, kind="ExternalInput")
    d_cvec = nc.dram_tensor("cvec", [128, 16], BF16, kind="ExternalInput")
    d_post = nc.dram_tensor("posT", [3, TOK], F32, kind="ExternalInput")
    d_fmat = nc.dram_tensor("fmat", [128, 128], F32, kind="ExternalInput")
    d_onespad = nc.dram_tensor("onespad", [128, 128], F32,
                               kind="ExternalInput")
    d_ident = nc.dram_tensor("ident", [128, 128], F32, kind="ExternalInput")
    d_pmat = nc.dram_tensor("pmat", [128, 128], BF16, kind="ExternalInput")
    d_ones128 = nc.dram_tensor("ones128", [1, 128], F32, kind="ExternalInput")
    d_ones128b = nc.dram_tensor("ones128b", [1, 128], BF16, kind="ExternalInput")
    d_onescol = nc.dram_tensor("onescol", [128, 1], F32, kind="ExternalInput")
    d_onescolb = nc.dram_tensor("onescolb", [128, 1], BF16,
                                kind="ExternalInput")
    d_qkmask = nc.dram_tensor("qkmask", [128, 8], BF16, kind="ExternalInput")
    d_selmask = nc.dram_tensor("selmask", [128, 256], BF16,
                               kind="ExternalInput")
    d_qsbias = nc.dram_tensor("qsbias", [4, 1], F32, kind="ExternalInput")
    d_msel = nc.dram_tensor("msel", [128, 2], F32, kind="ExternalInput")
    d_n2w = nc.dram_tensor("n2w", [128, 8], F32, kind="ExternalInput")
    d_out = nc.dram_tensor("out", [HID, OWN], F32, kind="ExternalOutput")

    with tile.TileContext(nc) as tc:
        _emit(nc, tc, locals())
    nc.compile()
    return nc


def _emit(nc, tc, d):
    d_xt = d["d_xt"]; d_xt_own = d["d_xt_own"]; d_qkw = d["d_qkw"]
    d_vw = d["d_vw"]; d_projw = d["d_projw"]; d_projb = d["d_projb"]
    d_w1 = d["d_w1"]; d_w3 = d["d_w3"]; d_w2 = d["d_w2"]
    d_adaw = d["d_adaw"]; d_adab = d["d_adab"]; d_cvec = d["d_cvec"]
    d_post = d["d_post"]; d_fmat = d["d_fmat"]; d_ones128 = d["d_ones128"]
    d_ones128b = d["d_ones128b"]; d_onescol = d["d_onescol"]
    d_onescolb = d["d_onescolb"]
    d_pmat = d["d_pmat"]; d_onespad = d["d_onespad"]
    d_ident = d["d_ident"]
    d_qkmask = d["d_qkmask"]; d_selmask = d["d_selmask"]
    d_qsbias = d["d_qsbias"]; d_msel = d["d_msel"]; d_n2w = d["d_n2w"]
    d_out = d["d_out"]

    sync = nc.sync
    act = nc.scalar
    dve = nc.vector
    pe = nc.tensor
    gps = nc.gpsimd

    with tc.tile_pool(name="dram", bufs=1, space="DRAM") as dram, \
         tc.tile_pool(name="p0", bufs=1) as p0, \
         tc.tile_pool(name="plate", bufs=1) as plate:

        # ---- dram bounce buffers for collectives ----
        ag_in = dram.tile([1536], F32)
        ag_out = dram.tile([1536 * N_CORES], F32, addr_space="Shared")
        a2_in = dram.tile([N_CORES, 128, OWN], BF16)
        a2_out = dram.tile([N_CORES, 128, OWN], BF16)

        # ---- small persistent constants ----
        cvec = p0.tile([128, 16], BF16)
        ones128 = p0.tile([1, 128], F32)
        ones128b = p0.tile([1, 128], BF16)
        onescol = p0.tile([128, 1], F32)
        onescolb = p0.tile([128, 1], BF16)
        sync.dma_start(out=onescolb[:], in_=d_onescolb[:])
        qkmask = p0.tile([128, 8], BF16)
        selmask = p0.tile([128, 256], BF16)
        onespad = p0.tile([128, 128], F32)
        sync.dma_start(out=onespad[:], in_=d_onespad[:])
        ident = p0.tile([128, 128], F32)
        sync.dma_start(out=ident[:], in_=d_ident[:])
        qsbias = p0.tile([4, 1], F32)
        msel = p0.tile([128, 2], F32)
        adab = p0.tile([1, 768], F32)
        projb = p0.tile([128, 8], F32)
        n2w = p0.tile([128, 8], F32)
        fmat = p0.tile([128, 128], F32)
        pmat = p0.tile([128, 128], BF16)
        sync.dma_start(out=pmat[:], in_=d_pmat[:])
        sync.dma_start(out=cvec[:], in_=d_cvec[:])
        sync.dma_start(out=ones128[:], in_=d_ones128[:])
        sync.dma_start(out=ones128b[:], in_=d_ones128b[:])
        sync.dma_start(out=onescol[:], in_=d_onescol[:])
        sync.dma_start(out=qkmask[:], in_=d_qkmask[:])
        sync.dma_start(out=selmask[:], in_=d_selmask[:])
        sync.dma_start(out=qsbias[:], in_=d_qsbias[:])
        sync.dma_start(out=msel[:], in_=d_msel[:])
        sync.dma_start(out=adab[:], in_=d_adab[:])
        sync.dma_start(out=projb[:], in_=d_projb[:])
        sync.dma_start(out=n2w[:], in_=d_n2w[:])
        sync.dma_start(out=fmat[:], in_=d_fmat[:])

        # const tiles for activation biases (must be APs for non-Copy funcs)
        cpi2 = p0.tile([128, 1], F32)
        dve.memset(cpi2[:], float(np.pi / 2))
        ceps = p0.tile([128, 1], F32)
        dve.memset(ceps[:], EPS)

        # persistent mid-size state
        modall = p0.tile([128, 96], F32)
        mod_own = p0.tile([128, 48], F32)
        sc1n_mlp = p0.tile([128, 8], F32)
        gbp = p0.tile([128, 8], F32)

        # ---- late-phase persistent tiles (proj/ffn) ----
        x2 = [plate.tile([128, OWN], F32, name=f"x2_{m}", tag=f"x2_{m}")
              for m in range(KC)]
        xh2 = [plate.tile([128, OWN], BF16, name=f"xh2_{m}", tag=f"xh2_{m}")
               for m in range(KC)]
        ofull = [plate.tile([128, OWN], BF16, name=f"of_{m}", tag=f"of_{m}")
                 for m in range(KC)]
        gated = [plate.tile([128, OWN], BF16, name=f"g_{mi}", tag=f"g_{mi}")
                 for mi in range(MI)]

        def modcol(b, comp, cf):
            cc = 8 * comp + cf
            return 12 * (cc // 6) + 6 * b + (cc % 6)

        # =======================================================
        # Phase A: sin/cos, AdaLN matmul + AllGather
        # =======================================================
        with tc.tile_pool(name="p_att", bufs=1) as p_att:
            # q/k tiles (post-qkv, per batch) and v_tok tiles
            qt = [p_att.tile([128, N], BF16, name=f"qt{b}", tag=f"qt{b}")
                  for b in range(B)]
            kt = [p_att.tile([128, N], BF16, name=f"kt{b}", tag=f"kt{b}")
                  for b in range(B)]
            ktp1 = [p_att.tile([128, N], BF16, name=f"ktp1{b}",
                               tag=f"ktp1{b}") for b in range(B)]
            vtok = [[p_att.tile([128, 130], BF16, name=f"vt{b}_{t}",
                                tag=f"vt{b}_{t}") for t in range(16)]
                    for b in range(B)]

            with tc.tile_pool(name="p_ph1", bufs=1) as p_ph1, \
                 tc.psum_pool(name="ps1", bufs=1) as ps1:

                # sincos tiles (consumed by rope, freed before attention)
                sinf = [p_ph1.tile([128, N], BF16, name=f"sinf{b}",
                                   tag=f"sinf{b}") for b in range(B)]
                cosf = [p_ph1.tile([128, N], BF16, name=f"cosf{b}",
                                   tag=f"cosf{b}") for b in range(B)]

                # --- AdaLN: mod slice = adaw.T @ c (per batch) + bias ---
                # AdaLN computed transposed: out[p, 6b+c] so the
                # AllGather payload is partition-major (no transpose DMAs).
                msl = p_ph1.tile([1, 1536], F32, tag="msl")
                ada_ps = [[ps1.tile([1, 512], F32, name=f"adaps{b}_{n}",
                                    tag="row", bufs=4)
                           for n in range(2)] for b in range(B)]
                with tc.tile_pool(name="p_ada", bufs=1) as p_ada:
                    adaw_ring = [p_ada.tile([128, 768], BF16, name=f"adaw{k}",
                                            tag="adaw", bufs=3)
                                 for k in range(KC)]
                    for k in range(KC):
                        sync.dma_start(out=adaw_ring[k][:],
                                       in_=d_adaw[128 * k:128 * (k + 1), :])
                    for b in range(B):
                        for n in range(2):
                            lo, sz = (0, 512) if n == 0 else (512, 256)
                            for k in range(KC):
                                pe.matmul(ada_ps[b][n][:, 0:sz],
                                          cvec[:, 2 * k + b: 2 * k + b + 1],
                                          adaw_ring[k][:, lo:lo + sz],
                                          start=(k == 0), stop=(k == KC - 1))
                            dve.tensor_tensor(
                                out=msl[:, 768 * b + lo: 768 * b + lo + sz],
                                in0=ada_ps[b][n][:, 0:sz],
                                in1=adab[:, lo:lo + sz], op=OP.add)
                sync.dma_start(out=ag_in[:], in_=msl[:])
                gps.collective_compute(
                    "AllGather", OP.bypass, replica_groups=GROUPS,
                    ins=[ag_in.opt()], outs=[ag_out.opt()])
                # --- sin/cos (Sin table set loads first) ---
                for b in range(B):
                    for t in range(4):
                        pts = p_ph1.tile([128, 512], F32, tag="pts", bufs=2)
                        sync.dma_start(out=pts[0:3, :], in_=d_post[
                            :, b * N + 512 * t: b * N + 512 * (t + 1)])
                        ang = ps1.tile([128, 512], F32, tag="mm", bufs=4)
                        pe.matmul(ang[:], fmat[:], pts[:],
                                  start=True, stop=True)
                        sl = slice(512 * t, 512 * (t + 1))
                        act.activation(sinf[b][:, sl], ang[:], AF.Sin)
                        act.activation(cosf[b][:, sl], ang[:], AF.Sin,
                                       bias=cpi2[:])

                sc1_b = []
                shb_b = []

                # =======================================================
                # Phase B (per batch): norm1 -> x_hat -> qkv -> rope
                # =======================================================
                qkw_sb = [p_ph1.tile([128, 256], BF16, name=f"qkw{k}",
                                     tag=f"qkw{k}") for k in range(KC)]
                vw_sb = [p_ph1.tile([128, 130], BF16, name=f"vw{k}",
                                    tag=f"vw{k}") for k in range(KC)]
                for k in range(KC):
                    sync.dma_start(out=qkw_sb[k][:],
                                   in_=d_qkw[128 * k:128 * (k + 1), :])
                    sync.dma_start(out=vw_sb[k][:],
                                   in_=d_vw[128 * k:128 * (k + 1), :])

                with tc.tile_pool(name="p_x", bufs=1) as p_x:
                    qkw_b = [[p_x.tile([128, 256], BF16, name=f"qkwb{b}_{k}",
                                       tag=f"qkwb{b}_{k}") for k in range(KC)]
                             for b in range(B)]
                    vw_b = [[p_x.tile([128, 130], BF16, name=f"vwb{b}_{k}",
                                      tag=f"vwb{b}_{k}") for k in range(KC)]
                            for b in range(B)]
                    qkb = [p_x.tile([128, 2], F32, name=f"qkb{b}",
                                    tag=f"qkb{b}") for b in range(B)]
                    vb_sb = [p_x.tile([1, 130], BF16, name=f"vb{b}",
                                      tag=f"vb{b}") for b in range(B)]

                    def emit_mod_and_folds():
                        # mod swizzle: deferred so early norm1 PE work covers
                        # the AllGather latency
                        ag4 = ag_out.rearrange("(q x p) -> q x p",
                                               q=N_CORES, x=12)
                        for q in range(N_CORES):
                            t12 = p_ph1.tile([12, 128], F32, tag="t12",
                                             bufs=2)
                            sync.dma_start(out=t12[:], in_=ag4[q])
                            tps = ps1.tile([128, 12], F32, tag="row",
                                           bufs=4)
                            pe.transpose(tps[:], t12[:], ident[0:12, 0:12])
                            dve.tensor_copy(
                                out=modall[:, 12 * q:12 * (q + 1)],
                                in_=tps[:])

                        # mod_own = blend of batch columns by msel
                        t_b0 = p_ph1.tile([128, 48], F32, tag="t_b0")
                        mv = modall.rearrange("p (q x) -> p q x", q=N_CORES)
                        for bb in range(B):
                            srcb = mv[:, :, 6 * bb:6 * bb + 6]
                            if bb == 0:
                                dve.tensor_scalar(
                                    t_b0[:].rearrange("p (q x) -> p q x",
                                                      q=8),
                                    srcb, msel[:, 0:1], None, OP.mult)
                            else:
                                dve.tensor_scalar(
                                    mod_own[:].rearrange("p (q x) -> p q x",
                                                         q=8),
                                    srcb, msel[:, 1:2], None, OP.mult)
                        dve.tensor_tensor(out=mod_own[:], in0=mod_own[:],
                                          in1=t_b0[:], op=OP.add)
                        dve.tensor_scalar(sc1n_mlp[:], mod_own[:, 32:40],
                                          1.0, None, OP.add)
                        dve.tensor_tensor(out=sc1n_mlp[:], in0=sc1n_mlp[:],
                                          in1=n2w[:], op=OP.mult)
                        dve.tensor_tensor(out=gbp[:], in0=mod_own[:, 16:24],
                                          in1=projb[:], op=OP.mult)

                        for bb in range(B):
                            sv = p_ph1.tile([128, 8], F32, name=f"sc1_{bb}",
                                            tag=f"sc1_{bb}")
                            hv = p_ph1.tile([128, 8], BF16, name=f"shb_{bb}",
                                            tag=f"shb_{bb}")
                            for cf in range(KC):
                                c_sc = modcol(bb, 1, cf)
                                dve.tensor_scalar(sv[:, cf:cf + 1],
                                                  modall[:, c_sc:c_sc + 1],
                                                  1.0, None, OP.add)
                                c_sh = modcol(bb, 0, cf)
                                dve.tensor_copy(out=hv[:, cf:cf + 1],
                                                in_=modall[:, c_sh:c_sh + 1])
                            sc1_b.append(sv)
                            shb_b.append(hv)

                        for bb in range(B):
                            # fold (1+sc)*norm1_w into weights; biases from
                            # unscaled weights x sh
                            for k in range(KC):
                                dve.tensor_scalar(qkw_b[bb][k][:],
                                                  qkw_sb[k][:],
                                                  sc1_b[bb][:, k:k + 1],
                                                  None, OP.mult)
                                dve.tensor_scalar(vw_b[bb][k][:], vw_sb[k][:],
                                                  sc1_b[bb][:, k:k + 1],
                                                  None, OP.mult)
                            for m in range(2):
                                bps = ps1.tile([128, 1], F32, tag="mm",
                                               bufs=4)
                                for k in range(KC):
                                    pe.matmul(
                                        bps[:],
                                        qkw_sb[k][:, 128 * m:128 * (m + 1)],
                                        shb_b[bb][:, k:k + 1],
                                        start=(k == 0), stop=(k == KC - 1))
                                act.activation(qkb[bb][:, m:m + 1], bps[:],
                                               AF.Copy)
                            vbp = ps1.tile([1, 130], F32, tag="row", bufs=4)
                            for k in range(KC):
                                pe.matmul(vbp[:], shb_b[bb][:, k:k + 1],
                                          vw_sb[k][:],
                                          start=(k == 0), stop=(k == KC - 1))
                            act.activation(vb_sb[bb][:], vbp[:], AF.Copy)

                    def norm1_t(b, t):
                        gsl = slice(b * N + 512 * t, b * N + 512 * (t + 1))
                        xts = [p_x.tile([128, 512], BF16, name=f"xt_{f}",
                                        tag="xtr", bufs=9)
                               for f in range(KC)]
                        ss = ps1.tile([1, 512], F32, tag="row", bufs=4)
                        for f in range(KC):
                            eng = sync if f % 2 == 0 else act
                            eng.dma_start(out=xts[f][:], in_=d_xt[
                                128 * f:128 * (f + 1), gsl])
                            sq = p_x.tile([128, 512], BF16, tag="sq", bufs=2)
                            dve.tensor_tensor(out=sq[:], in0=xts[f][:],
                                              in1=xts[f][:], op=OP.mult)
                            pe.matmul(ss[:], onescolb[:], sq[:],
                                      start=(f == 0), stop=(f == KC - 1))
                        lnv = p_x.tile([1, 512], F32, tag="lnv", bufs=1)
                        act.activation(lnv[:], ss[:], AF.Ln, bias=ceps[0:1, :],
                                       scale=float(1.0 / HID))
                        rstd = p_x.tile([128, 512], F32, tag="rstd", bufs=1)
                        act.activation(rstd[0:1, :], lnv[:], AF.Exp,
                                       scale=-0.5)
                        bc_ps = ps1.tile([128, 512], F32, tag="mm", bufs=4)
                        pe.matmul(bc_ps[:], onespad[:], rstd[:],
                                  start=True, stop=True)
                        bc = p_x.tile([128, 512], BF16, tag="bc", bufs=2)
                        act.activation(bc[:], bc_ps[:], AF.Copy)
                        xh = [p_x.tile([128, 512], BF16, name=f"xh{b}{t}_{f}",
                                       tag=f"xh_{f}", bufs=3)
                              for f in range(KC)]
                        for f in range(KC):
                            dve.tensor_tensor(out=xh[f][:], in0=xts[f][:],
                                              in1=bc[:], op=OP.mult)
                        return xh

                    def qkv_t(b, t, xh):
                        tsl = slice(512 * t, 512 * (t + 1))
                        for m in range(2):
                            dst = qt[b] if m == 0 else kt[b]
                            ps = ps1.tile([128, 512], F32, tag="mm", bufs=4)
                            for k in range(KC):
                                pe.matmul(ps[:],
                                          qkw_b[b][k][:, 128 * m:
                                                      128 * (m + 1)],
                                          xh[k][:],
                                          start=(k == 0), stop=(k == KC - 1))
                            act.activation(dst[:, tsl], ps[:], AF.Identity,
                                           bias=qkb[b][:, m:m + 1])
                        for c4 in range(4):
                            tb = 4 * t + c4
                            ps = ps1.tile([128, 130], F32, tag="mm", bufs=4)
                            for k in range(KC):
                                pe.matmul(ps[:],
                                          xh[k][:, 128 * c4:128 * (c4 + 1)],
                                          vw_b[b][k][:],
                                          start=(k == 0), stop=False)
                            pe.matmul(ps[:], ones128b[:], vb_sb[b][:],
                                      start=False, stop=True)
                            dve.tensor_copy(out=vtok[b][tb][:], in_=ps[:])
                            vv = vtok[b][tb].rearrange("p (h x) -> p h x", h=2)
                            dve.memset(vv[:, :, 64:65], 1.0)

                    for b in range(B):
                        pend = []
                        for t in range(4):
                            if b == 0 and t == 2:
                                emit_mod_and_folds()
                            xh = norm1_t(b, t)
                            pend.append((t, xh))
                            if len(pend) > 2:
                                tt, xx = pend.pop(0)
                                qkv_t(b, tt, xx)
                        for tt, xx in pend:
                            qkv_t(b, tt, xx)

                        # ---- q/k rmsnorm (joint, partition-reduced) ----
                        rstd8 = p_x.tile([128, N], BF16, tag="rstd8")
                        dve.memset(rstd8[:], 0.0)
                        ss8 = [ps1.tile([4, 512], F32, name=f"ss8_{b}_{t}",
                                        tag="row", bufs=4) for t in range(4)]
                        for m in range(2):
                            src = qt[b] if m == 0 else kt[b]
                            for t in range(4):
                                sqm = p_x.tile([128, 512], BF16, tag="sqm",
                                               bufs=2)
                                sl = slice(512 * t, 512 * (t + 1))
                                dve.tensor_tensor(out=sqm[:], in0=src[:, sl],
                                                  in1=src[:, sl], op=OP.mult)
                                pe.matmul(ss8[t][:],
                                          qkmask[:, 4 * m:4 * (m + 1)],
                                          sqm[:],
                                          start=(m == 0), stop=(m == 1))
                        for t in range(4):
                            l8 = p_x.tile([4, 512], F32, tag="l8", bufs=1)
                            act.activation(l8[:], ss8[t][:], AF.Ln,
                                           bias=ceps[0:4, :],
                                           scale=float(1.0 / HD))
                            act.activation(rstd8[0:4, 512 * t:512 * (t + 1)],
                                           l8[:], AF.Exp, scale=-0.5,
                                           bias=qsbias[:])
                        # apply rstd (* q_norm_w/k_norm_w via selmask)
                        for m in range(2):
                            dst = qt[b] if m == 0 else kt[b]
                            for t in range(4):
                                bcp = ps1.tile([128, 512], F32, tag="mm",
                                               bufs=4)
                                pe.matmul(bcp[:],
                                          selmask[:, 128 * m:128 * (m + 1)],
                                          rstd8[:, 512 * t:512 * (t + 1)],
                                          start=True, stop=True)
                                sl = slice(512 * t, 512 * (t + 1))
                                dve.tensor_tensor(out=dst[:, sl],
                                                  in0=dst[:, sl],
                                                  in1=bcp[:], op=OP.mult)

                        # ---- rope (partner shuffle via PE permutation) ----
                        for m in range(2):
                            src = qt[b] if m == 0 else kt[b]
                            for t in range(4):
                                sl = slice(512 * t, 512 * (t + 1))
                                pps = ps1.tile([128, 512], F32, tag="mm",
                                               bufs=4)
                                pe.matmul(pps[:], pmat[:], src[:, sl],
                                          start=True, stop=True)
                                part = p_x.tile([128, 512], BF16, tag="part",
                                                bufs=2)
                                dve.tensor_tensor(out=part[:], in0=pps[:],
                                                  in1=sinf[b][:, sl],
                                                  op=OP.mult)
                                dve.tensor_tensor(out=src[:, sl],
                                                  in0=src[:, sl],
                                                  in1=cosf[b][:, sl],
                                                  op=OP.mult)
                                dve.tensor_tensor(out=src[:, sl],
                                                  in0=src[:, sl],
                                                  in1=part[:], op=OP.add)

                        # zero-padded key tiles: scores contract K=128 with
                        # the unused half of the array masked by zero rows
                        # (K<=64 matmuls run at half rate and never ramp).
                        dve.tensor_copy(out=ktp1[b][64:128, :],
                                        in_=kt[b][64:128, :])
                        dve.memset(ktp1[b][0:64, :], 0.0)
                        dve.memset(kt[b][64:128, :], 0.0)

            # =======================================================
            # Phase C: attention (per batch, per local head, per half)
            # =======================================================
            with tc.tile_pool(name="p_att2", bufs=1) as p_at2:
                psA = tc.alloc_tile_pool(name="ps_att", bufs=1, space="PSUM")

                # prefetch FFN w1/w3 as mi-major rings; transfers hide
                # under attention, ring refills during the gated loop
                w1t = [p_at2.tile([128, HID], BF16, name=f"w1_{mi}",
                                  tag="w1r", bufs=8) for mi in range(MI)]
                w3t = [p_at2.tile([128, HID], BF16, name=f"w3_{mi}",
                                  tag="w3r", bufs=8) for mi in range(MI)]
                for mi in range(8):
                    sync.dma_start(out=w1t[mi][:],
                                   in_=d_w1[128 * mi:128 * (mi + 1), :])
                    sync.dma_start(out=w3t[mi][:],
                                   in_=d_w3[128 * mi:128 * (mi + 1), :])

                def normalize(oT, b, hl, half):
                    # softmax normalize: divide rows 0-63 by the ones-row
                    # (row 64).  DVE reciprocal + PE broadcast; no ACT work
                    # so the Exp stream of the next block is undisturbed.
                    hsl = slice(64 * hl, 64 * (hl + 1))
                    rinv = p_at2.tile([128, 1024], F32, tag="rinv", bufs=2)
                    dve.reciprocal(rinv[0:1, :], oT[64:65, :])
                    bcp = psA.tile([64, 1024], F32, tag="sc", bufs=2)
                    for q2 in range(2):
                        pe.matmul(bcp[:, 512 * q2:512 * (q2 + 1)],
                                  onespad[:, 0:64],
                                  rinv[:, 512 * q2:512 * (q2 + 1)],
                                  start=True, stop=True)
                    bc_sb = p_at2.tile([64, 1024], F32, tag="bcs", bufs=2)
                    dve.tensor_copy(out=bc_sb[:], in_=bcp[:])
                    osb = p_at2.tile([64, 1024], BF16, tag="osb", bufs=2)
                    dve.tensor_tensor(out=osb[:], in0=oT[0:64, :],
                                      in1=bc_sb[:], op=OP.mult)
                    for s2 in range(2):
                        sync.dma_start(
                            out=a2_in[4 * b + 2 * half + s2, hsl, :],
                            in_=osb[:, 512 * s2:512 * (s2 + 1)])

                pending = None
                for b in range(B):
                    for hl in range(2):
                        hsl = slice(64 * hl, 64 * (hl + 1))
                        for half in range(2):
                            oT = psA.tile([65, 1024], F32, tag="oT", bufs=2)
                            for nk in range(16):
                                if nk == 6 and pending is not None:
                                    normalize(*pending)
                                    pending = None
                                sc = psA.tile([128, 1024], F32, tag="sc", bufs=2)
                                ksrc = kt[b] if hl == 0 else ktp1[b]
                                for q2 in range(2):
                                    qsl = slice(1024 * half + 512 * q2,
                                                1024 * half + 512 * (q2 + 1))
                                    pe.matmul(sc[:, 512 * q2:512 * (q2 + 1)],
                                              ksrc[:, 128 * nk:128 * (nk + 1)],
                                              qt[b][:, qsl],
                                              start=True, stop=True)
                                at = p_at2.tile([128, 1024], BF16, tag="at",
                                                bufs=3)
                                act.activation(at[:], sc[:], AF.Exp)
                                for q2 in range(2):
                                    pe.matmul(
                                        oT[:, 512 * q2:512 * (q2 + 1)],
                                        vtok[b][nk][:, 65 * hl:65 * (hl + 1)],
                                        at[:, 512 * q2:512 * (q2 + 1)],
                                        start=(nk == 0), stop=(nk == 15))
                            pending = (oT, b, hl, half)
                normalize(*pending)

                # ---- AllToAll: head-major -> token-major ----
                gps.collective_compute(
                    "AllToAll", OP.bypass, replica_groups=GROUPS,
                    ins=[a2_in.opt()], outs=[a2_out.opt()])
                for k in range(KC):
                    sync.dma_start(out=ofull[k][:], in_=a2_out[k])

                # ---- proj + gated residual -> x2 (fp32) ----
                projw_sb = [p_at2.tile([128, HID], BF16, name=f"pw_{k}",
                                       tag=f"pw_{k}") for k in range(KC)]
                for m in range(KC):
                    sync.dma_start(out=x2[m][:],
                                   in_=d_xt_own[128 * m:128 * (m + 1), :])
                for k in range(KC):
                    sync.dma_start(out=projw_sb[k][:],
                                   in_=d_projw[128 * k:128 * (k + 1), :])
                psA.release()
                with tc.psum_pool(name="ps_pr", bufs=1) as psP:
                    for m in range(KC):
                        ps = psP.tile([128, 512], F32, tag="mm", bufs=4)
                        for k in range(KC):
                            pe.matmul(ps[:], projw_sb[k][:, 128 * m:128 * (m + 1)],
                                      ofull[k][:], start=(k == 0),
                                      stop=(k == KC - 1))
                        tg = p_at2.tile([128, OWN], F32, tag="tg", bufs=2)
                        act.activation(tg[:], ps[:], AF.Identity,
                                       bias=gbp[:, m:m + 1],
                                       scale=mod_own[:, 16 + m:17 + m])
                        dve.tensor_tensor(out=x2[m][:], in0=x2[m][:],
                                          in1=tg[:], op=OP.add)

                    # ---- norm2 + modulate -> xh2 (bf16) ----
                    ss2 = psP.tile([1, 512], F32, tag="row", bufs=2)
                    for m in range(KC):
                        sq = p_at2.tile([128, OWN], F32, tag="sq2", bufs=2)
                        dve.tensor_tensor(out=sq[:], in0=x2[m][:], in1=x2[m][:],
                                          op=OP.mult)
                        pe.matmul(ss2[:], onescol[:], sq[:],
                                  start=(m == 0), stop=(m == KC - 1))
                    ln2 = p_at2.tile([1, 512], F32, tag="ln2")
                    act.activation(ln2[:], ss2[:], AF.Ln, bias=ceps[0:1, :],
                                   scale=float(1.0 / HID))
                    rstd2 = p_at2.tile([128, 512], F32, tag="rstd2")
                    act.activation(rstd2[0:1, :], ln2[:], AF.Exp, scale=-0.5)
                    bc2p = psP.tile([128, 512], F32, tag="mm", bufs=4)
                    pe.matmul(bc2p[:], onespad[:], rstd2[:], start=True,
                              stop=True)
                    bc2 = p_at2.tile([128, 512], F32, tag="bc2")
                    act.activation(bc2[:], bc2p[:], AF.Copy)
                    for m in range(KC):
                        tmp = p_at2.tile([128, OWN], F32, tag="tmp2", bufs=2)
                        dve.tensor_tensor(out=tmp[:], in0=x2[m][:], in1=bc2[:],
                                          op=OP.mult)
                        dve.tensor_scalar(xh2[m][:], tmp[:],
                                          sc1n_mlp[:, m:m + 1],
                                          mod_own[:, 24 + m:25 + m],
                                          OP.mult, OP.add)

                    # ---- FFN w1/w3 + swiglu gate (weights prefetched) ----
                    for mi in range(MI):
                        if mi + 8 < MI:
                            sync.dma_start(
                                out=w1t[mi + 8][:],
                                in_=d_w1[128 * (mi + 8):128 * (mi + 9), :])
                            sync.dma_start(
                                out=w3t[mi + 8][:],
                                in_=d_w3[128 * (mi + 8):128 * (mi + 9), :])
                        p1 = psP.tile([128, 512], F32, tag="mm", bufs=4)
                        p3 = psP.tile([128, 512], F32, tag="mm", bufs=4)
                        for k in range(KC):
                            pe.matmul(p1[:],
                                      w1t[mi][:, 128 * k:128 * (k + 1)],
                                      xh2[k][:], start=(k == 0),
                                      stop=(k == KC - 1))
                        for k in range(KC):
                            pe.matmul(p3[:],
                                      w3t[mi][:, 128 * k:128 * (k + 1)],
                                      xh2[k][:], start=(k == 0),
                                      stop=(k == KC - 1))
                        s1 = p_at2.tile([128, OWN], BF16, tag="s1", bufs=3)
                        act.activation(s1[:], p1[:], AF.Silu)
                        dve.tensor_tensor(out=gated[mi][:], in0=s1[:],
                                          in1=p3[:], op=OP.mult)

        # =======================================================
        # Phase D: w2 + final residual
        # =======================================================
        with tc.tile_pool(name="p_ffn", bufs=1) as p_ffn:
            with tc.psum_pool(name="ps_f2", bufs=1) as psW:
                ffp = [psW.tile([128, 512], F32, name=f"ff_{m}", tag=f"ff_{m}")
                       for m in range(KC)]
                w2r = [p_ffn.tile([128, HID], BF16, name=f"w2_{mi}", tag="w2r",
                                  bufs=8) for mi in range(MI)]
                for mi in range(MI):
                    sync.dma_start(out=w2r[mi][:],
                                   in_=d_w2[128 * mi:128 * (mi + 1), :])
                    for m in range(KC):
                        pe.matmul(ffp[m][:], w2r[mi][:, 128 * m:128 * (m + 1)],
                                  gated[mi][:], start=(mi == 0),
                                  stop=(mi == MI - 1))
                for m in range(KC):
                    tg = p_ffn.tile([128, OWN], F32, tag="tgo", bufs=2)
                    act.activation(tg[:], ffp[m][:], AF.Identity, bias=0.0,
                                   scale=mod_own[:, 40 + m:41 + m])
                    outm = p_ffn.tile([128, OWN], F32, tag="outm", bufs=2)
                    dve.tensor_tensor(out=outm[:], in0=tg[:], in1=x2[m][:],
                                      op=OP.add)
                    sync.dma_start(out=d_out[128 * m:128 * (m + 1), :],
                                   in_=outm[:])


def _host_prep(x, c, positions, norm1_w, qkv_w, q_norm_w, k_norm_w, proj_w,
               proj_b, norm2_w, w1, w3, w2, ada_w, ada_b):
    xf = np.asarray(x, np.float32).reshape(TOK, HID)
    xt = np.ascontiguousarray(xf.T)                      # [1024, 4096]
    xtb = xt.astype(ml_dtypes.bfloat16)
    perm = np.array(_perm())
    freqs = _freqs()

    qkv_r = np.asarray(qkv_w, np.float32).reshape(3, HEADS, HD, HID)
    n1 = np.asarray(norm1_w, np.float32)
    # fmat: [128, 128] angle matrix (rows 0-2 live, rest zero so the
    # device-side rhs tiles only need their top rows filled)
    fmat = np.zeros((128, 128), np.float32)
    for r in range(128):
        rr = r % 64
        if rr < 30:
            j = rr
            fmat[j // 10, r] = -freqs[j % 10]
        elif rr >= 34:
            j = rr - 34
            fmat[j // 10, r] = freqs[j % 10]
    posT = np.ascontiguousarray(
        np.asarray(positions, np.float32).reshape(TOK, 3).T)

    # rope partner-shuffle as a PE permutation: out[d] = src[perm_src(d)]
    # lhsT layout: pmat[src_row, dst_row] = 1
    pmat = np.zeros((128, 128), np.float32)
    for dlo, slo, cnt in ((0, 34, 30), (34, 0, 30), (64, 98, 30),
                          (98, 64, 30), (30, 30, 4), (94, 94, 4)):
        for r in range(cnt):
            pmat[slo + r, dlo + r] = 1.0

    qkmask = np.zeros((128, 8), np.float32)
    qkmask[0:64, 0] = 1.0
    qkmask[64:128, 1] = 1.0
    qkmask[0:64, 6] = 1.0
    qkmask[64:128, 7] = 1.0
    # selmask carries q_norm_w / k_norm_w (permuted) in place of 1.0
    qnw = np.asarray(q_norm_w, np.float32)[perm]
    knw = np.asarray(k_norm_w, np.float32)[perm]
    selmask = np.zeros((128, 256), np.float32)
    for p in range(128):
        selmask[p // 64, p] = qnw[p % 64]
        selmask[2 + p // 64, 128 + p] = knw[p % 64]
    qsbias = np.array([[np.log(HD ** -0.5)]] * 2 + [[0.0]] * 2, np.float32)

    projw = np.ascontiguousarray(
        np.asarray(proj_w, np.float32).T * 1.0).astype(ml_dtypes.bfloat16)
    projb = np.asarray(proj_b, np.float32).reshape(8, 128).T.copy()
    n2w = (np.asarray(norm2_w, np.float32)).reshape(8, 128).T.copy()

    w1p = np.zeros((SWIGLU_P, HID), np.float32)
    w1p[:SWIGLU] = np.asarray(w1, np.float32)
    w3p = np.zeros((SWIGLU_P, HID), np.float32)
    w3p[:SWIGLU] = np.asarray(w3, np.float32)
    w2p = np.zeros((SWIGLU_P, HID), np.float32)
    w2p[:SWIGLU] = np.asarray(w2, np.float32).T
    def mi_major(w):  # [SWIGLU_P, HID] -> tile[mi][p, 128k+j] = w[128mi+j, 128k+p]
        w4 = w.reshape(MI, 128, KC, 128)          # [mi, j, k, p]
        w4 = np.ascontiguousarray(w4.transpose(0, 3, 2, 1))  # [mi, p, k, j]
        return w4.reshape(SWIGLU_P, HID)

    w1w = mi_major(w1p).astype(ml_dtypes.bfloat16)
    w3w = mi_major(w3p).astype(ml_dtypes.bfloat16)
    w2w = w2p.astype(ml_dtypes.bfloat16)

    adaw_t = np.ascontiguousarray(np.asarray(ada_w, np.float32).T)  # [1024, 6144]
    adab = np.asarray(ada_b, np.float32)
    cvec = np.zeros((128, 16), np.float32)
    cf = np.asarray(c, np.float32)
    for k in range(KC):
        for b in range(B):
            cvec[:, 2 * k + b] = cf[b, 128 * k:128 * (k + 1)]

    ones128 = np.ones((1, 128), np.float32)
    onescol = np.ones((128, 1), np.float32)
    onespad = np.zeros((128, 128), np.float32)
    onespad[0, :] = 1.0
    ident = np.eye(128, dtype=np.float32)

    common = dict(
        xt=xtb,
        projw=projw, projb=projb, n2w=n2w,
        w1w=w1w, w3w=w3w, w2w=w2w,
        cvec=cvec.astype(ml_dtypes.bfloat16),
        posT=posT, fmat=fmat, pmat=pmat.astype(ml_dtypes.bfloat16),
        ones128=ones128, ones128b=ones128.astype(ml_dtypes.bfloat16),
        onescol=onescol, onescolb=onescol.astype(ml_dtypes.bfloat16),
        onespad=onespad, ident=ident,
        qkmask=qkmask.astype(ml_dtypes.bfloat16),
        selmask=selmask.astype(ml_dtypes.bfloat16), qsbias=qsbias,
    )

    in_maps = []
    for i in range(N_CORES):
        m = dict(common)
        h0 = 2 * i
        q = qkv_r[0, h0:h0 + 2][:, perm, :]       # [2, 64, 1024]
        k = qkv_r[1, h0:h0 + 2][:, perm, :]
        v = qkv_r[2, h0:h0 + 2]                   # unpermuted
        qk = np.concatenate([q.reshape(128, HID), k.reshape(128, HID)], 0)
        qk = qk * n1[None, :]                     # fold norm1_w
        m["qkw"] = np.ascontiguousarray(qk.T).astype(ml_dtypes.bfloat16)
        vw = np.zeros((HID, 130), np.float32)
        for hh in range(2):
            vw[:, 65 * hh:65 * hh + 64] = (v[hh] * n1[None, :]).T
        m["vw"] = vw.astype(ml_dtypes.bfloat16)
        m["adaw"] = adaw_t[:, 768 * i:768 * (i + 1)].astype(ml_dtypes.bfloat16)
        m["adab"] = adab[768 * i:768 * (i + 1)].reshape(1, 768).copy()
        m["xt_own"] = np.ascontiguousarray(xt[:, OWN * i:OWN * (i + 1)])
        msel = np.zeros((128, 2), np.float32)
        own_batch = (OWN * i) // N
        msel[:, 0] = 1.0 - own_batch
        msel[:, 1] = float(own_batch)
        m["msel"] = msel
        in_maps.append(m)
    return in_maps


def kernel(**inputs) -> np.ndarray:
    if "nc" not in _cache:
        _cache["nc"] = build_program()
    nc = _cache["nc"]
    in_maps = _host_prep(**inputs)
    res = run_bass_kernel_spmd(nc, in_maps, core_ids=list(range(N_CORES)))
    out = np.empty((TOK, HID), np.float32)
    for i in range(N_CORES):
        out[OWN * i:OWN * (i + 1), :] = res.results[i]["out"].T
    return out.reshape(B, N, HID)

